# revision 22
# baseline (speedup 1.0000x reference)
"""Trainium2 Bass kernel for quaternion capsule routing layer.

Math (reference):
  qn = normalize(quats); votes[n,o,i,:] = scale[o,i]*H(qn[o,i], x[n,i]) + (0,trans)
  3 iterations of dynamic routing (softmax over o, weighted vote sum, squash,
  agreement update), then sigmoid-gated output poses.

Strategy (8 cores, data-parallel over n):
  - votes are a LINEAR map of x: host folds rotation+scale+translation into
    W [65, 1088] (64 x-features + ones row).  Columns 0..1023 = votes in
    (d, o, i) order, columns 1024..1087 = iteration-1 s (c is uniform 1/16).
  - Per 128-capsule tile: PE transposes x, float32r matmuls produce votes in
    PSUM; routing runs on DVE (custom fused multiply+prefix-scan op for the
    two big einsums, segment sums extracted by strided subtracts) and ACT
    (exp/ln only -> single activation table set, no table switches).
"""

import numpy as np

import concourse.bass as bass
import concourse.mybir as mybir
import concourse.dve_ops as dve_ops
from concourse.dve_spec import Spec, Src0, Src1, scan, AluOp, lower, _has_src1
from concourse.dve_uop import DveOpSpec
from concourse.tile import TileContext
from concourse import bass_utils

F32 = mybir.dt.float32
F32R = mybir.dt.float32r
AX = mybir.AxisListType
OP = mybir.AluOpType
AF = mybir.ActivationFunctionType

N, I, O, ITERS = 65536, 16, 16, 3
EPS = 1e-8
NCORES = 8
P = 128
N_CORE = N // NCORES            # 8192
NT = N_CORE // P                # 64 tiles per core
NCHUNK = 4                      # x/y staged in 4 DMA chunks
TPC = NT // NCHUNK              # tiles per chunk


# --------------------------------------------------------------------------
# custom DVE op: out[p,k] = cumsum_k(in0[p,k] * in1[p,k])   (fp32, inclusive)
# --------------------------------------------------------------------------
def _mulscan_ref(in0, in1, s0, s1, imm2):
    p = in0.astype(np.float32) * np.asarray(in1, dtype=np.float32)
    flat = p.reshape(p.shape[0], -1)
    return np.cumsum(flat, axis=1, dtype=np.float32).reshape(in0.shape)


def _register_mulscan():
    name = "CAPS_MULSCAN_V1"
    for op in dve_ops.OPS:
        if op.name == name:
            return op
    row = dve_ops._CUSTOM_DVE_ROW_BASE + len(dve_ops.OPS)
    dve_ops._SUB_OPCODE_FOR_NAME[name] = row
    spec = Spec(body=scan(AluOp.ADD, Src0 * Src1), reference=_mulscan_ref)
    shas = {}
    for ver in ("v3", "v4"):
        try:
            s = DveOpSpec(name=name, opcode=row, uops=lower(spec, ver=ver),
                          rd1_en=_has_src1(spec))
            shas[ver] = s.sha(ver)
        except Exception:
            pass
    op = dve_ops.DveOp(name, spec, subdim=False, uops_sha=shas)
    dve_ops.OPS.append(op)
    dve_ops.CUSTOM_DVE_SPECS[name] = spec
    return op


MULSCAN = _register_mulscan()


# --------------------------------------------------------------------------
# host-side parameter folding
# --------------------------------------------------------------------------
def _build_W(quats, scale, trans):
    """W [65, 1088] f32.  Rows (i*4+e) for e in 0..3 plus ones-row 64.
    Cols 0..1023: votes, col j = d*256 + o*16 + i.
    Cols 1024..1087: iter-1 s (=mean of votes over i), col 1024 + d*16 + o."""
    q = quats.astype(np.float64)
    qn = q / np.sqrt((q * q).sum(-1, keepdims=True) + EPS)
    w, x, y, z = qn[..., 0], qn[..., 1], qn[..., 2], qn[..., 3]
    # Hamilton left-multiplication matrix M[d, e]: (q (x) r)_d = sum_e M_de r_e
    M = np.stack([
        np.stack([w, -x, -y, -z], -1),
        np.stack([x,  w, -z,  y], -1),
        np.stack([y,  z,  w, -x], -1),
        np.stack([z, -y,  x,  w], -1),
    ], -2)                                    # [O, I, d, e]
    A = scale.astype(np.float64)[..., None] * M     # [O, I, d, e]
    t = np.concatenate([np.zeros(trans.shape[:-1] + (1,)),
                        trans.astype(np.float64)], -1)   # [O, I, d]

    W = np.zeros((65, 1088), np.float64)
    # votes block
    Wv = W[:, :1024].reshape(65, 4, O, I)       # [row, d, o, i]
    for i in range(I):
        # rows i*4+e  ->  cols (d, o, i)
        Wv[i * 4:(i + 1) * 4, :, :, i] = A[:, i, :, :].transpose(2, 1, 0)  # [e,d,o]? fix below
    # careful: A[o,i,d,e] -> want Wv[i*4+e, d, o, i] = A[o,i,d,e]
    # A[:, i, :, :] is [o, d, e]; transpose(2,1,0) -> [e, d, o]  (correct)
    Wv[64, :, :, :] = t.transpose(2, 0, 1)      # [d, o, i] <- t[o,i,d]
    # s1 block: s1[d,o] = (1/16) * sum_i votes[d,o,i]
    Ws = W[:, 1024:].reshape(65, 4, O)
    for i in range(I):
        Ws[i * 4:(i + 1) * 4, :, :] += A[:, i, :, :].transpose(2, 1, 0) / I
    Ws[64, :, :] += t.transpose(2, 0, 1).sum(-1) / I
    return np.ascontiguousarray(W, dtype=np.float32)


# --------------------------------------------------------------------------
# device kernel
# --------------------------------------------------------------------------
def _strided(ap, off, dims):
    """AP at element offset `off` past `ap`'s origin with free dims
    [[step, count], ...] (partition dim copied from ap)."""
    return bass.AP(ap.tensor, ap.offset + off, [list(ap.ap[0])] + [list(d) for d in dims])


def _fixup_bir_for_walrus(nc):
    """Adapt Tile/bass output to this container's walrus build:
    1. Every ISA struct here has a single sync-wait slot; Tile can emit
       several waits on one instruction.  Move all-but-one wait onto NoOps
       inserted just before it in the same engine stream (equivalent:
       waits hold monotonically within an execution phase).
    2. EVENT_SEMAPHORE_RANGE_CLEAR (opcode 176) is unknown to this walrus;
       replace with per-semaphore writes of 0."""
    import bass_rust as _br
    cnt = 0
    for blk in nc.m.functions[0].blocks:
        out = []
        changed = False
        for ins in blk.instructions:
            si = ins.sync_info
            if si is not None and len(si.on_wait) > 1:
                waits = list(si.on_wait)
                for w in waits[:-1]:
                    cnt += 1
                    nop = mybir.InstNoOp(
                        name=f"I-wsplit-{cnt}", engine=ins.engine,
                        text_hint="wsplit", bass_nofuse=True,
                        ins=[], outs=[],
                        sync_info=_br.SyncInfo(on_wait=[w], on_update=[]))
                    out.append(nop)
                ins.sync_info = _br.SyncInfo(
                    on_wait=[waits[-1]], on_update=list(si.on_update))
                changed = True
            if (type(ins).__name__ == "InstISA"
                    and getattr(ins, "ant_dict", None)
                    and ins.ant_dict.get("header", {}).get("opcode") == 176):
                lo = ins.ant_dict["range_first"]
                hi = ins.ant_dict["range_last"]
                base_si = ins.sync_info
                for k, sem in enumerate(range(lo, hi + 1)):
                    cnt += 1
                    upd = _br.SyncUpdate(
                        sync_type="semaphore", id=sem,
                        update_mode="sem-wr-imm", update_value=0)
                    ev = mybir.InstEventSemaphore(
                        name=f"I-semclr-{cnt}", engine=ins.engine,
                        ins=[], outs=[],
                        sync_info=_br.SyncInfo(
                            on_wait=list(base_si.on_wait) if (k == 0 and base_si) else [],
                            on_update=[upd]))
                    out.append(ev)
                changed = True
                continue
            out.append(ins)
        if changed:
            blk.instructions = out
    return cnt


def _build_nc():
    nc = bass.Bass(trn_type="TRN2")
    x_d = nc.dram_tensor("x", [N_CORE, 65], F32, kind="ExternalInput")
    W_d = nc.dram_tensor("W", [65, 1088], F32, kind="ExternalInput")
    bb_d = nc.dram_tensor("bb", [P, O], F32, kind="ExternalInput")   # beta bcast
    ab_d = nc.dram_tensor("ab", [P, O], F32, kind="ExternalInput")   # alpha+bias bcast
    id_d = nc.dram_tensor("ident", [P, P], F32, kind="ExternalInput")
    y_d = nc.dram_tensor("y", [N_CORE, 64], F32, kind="ExternalOutput")

    with TileContext(nc) as tc, \
         tc.tile_pool(name="const", bufs=1) as cpool, \
         tc.tile_pool(name="stage", bufs=1) as spool, \
         tc.tile_pool(name="lhs", bufs=3) as lpool, \
         tc.tile_pool(name="big", bufs=2) as bigpool, \
         tc.tile_pool(name="sm", bufs=3) as smpool, \
         tc.tile_pool(name="pv", bufs=2, space="PSUM") as pv, \
         tc.tile_pool(name="px", bufs=2, space="PSUM") as px, \
         tc.tile_pool(name="ps1", bufs=2, space="PSUM") as ps1:

        W_sb = cpool.tile([65, 1088], F32, tag="W")
        id_sb = cpool.tile([P, P], F32, tag="ident")
        bb_sb = cpool.tile([P, O], F32, tag="bb")
        ab_sb = cpool.tile([P, O], F32, tag="ab")
        eps_sb = cpool.tile([P, 1], F32, tag="eps")
        nc.vector.memset(eps_sb[:, :], EPS)
        nc.sync.dma_start(out=W_sb[:, :], in_=W_d[:, :])
        nc.sync.dma_start(out=id_sb[:, :], in_=id_d[:, :])
        nc.sync.dma_start(out=bb_sb[:, :], in_=bb_d[:, :])
        nc.sync.dma_start(out=ab_sb[:, :], in_=ab_d[:, :])

        xs = spool.tile([P, NT * 65], F32, tag="xs")
        nc.sync.dma_start(
            out=xs[:, :].rearrange("p (t f) -> p t f", f=65),
            in_=x_d[:, :].rearrange("(t p) f -> p t f", p=P),
        )
        ys = []
        for j in range(NCHUNK):
            ys_j = spool.tile([P, TPC * 64], F32, tag=f"ys{j}")
            ys.append(ys_j)

        # Prologue: PE ops absorbing one DMA-lane wait each (the LDWEIGHTS
        # struct supports a single sync wait), so steady-state matmuls only
        # ever wait on the DVE semaphore.
        pa = px.tile([P, P], F32, tag="xt")
        nc.tensor.transpose(pa[:, :], id_sb[:, :], id_sb[:, :])
        pb = px.tile([P, P], F32, tag="xt")
        nc.tensor.transpose(pb[:, :], W_sb[:, 0:P], id_sb[0:65, :])
        pc = px.tile([P, P], F32, tag="xt")
        nc.tensor.transpose(pc[:65, :], xs[:, 0:65], id_sb[:, :])

        for t in range(NT):
            j, tt = divmod(t, TPC)
            # ---- PE: transpose x tile, matmul votes + s1 ----
            xt = px.tile([65, P], F32, tag="xt")
            nc.tensor.transpose(xt[:, :], xs[:, t * 65:(t + 1) * 65], id_sb[:, :])
            lhs = lpool.tile([65, P], F32, tag="lhs")
            nc.vector.tensor_copy(lhs[:, :], xt[:, :])

            votes = pv.tile([P, 1024], F32, tag="votes")
            s1 = ps1.tile([P, 64], F32, tag="s1")
            nc.tensor.matmul(votes[:, 0:512], lhs[:, :], W_sb[:, 0:512], start=True, stop=True)
            nc.tensor.matmul(votes[:, 512:1024], lhs[:, :], W_sb[:, 512:1024], start=True, stop=True)
            nc.tensor.matmul(s1[:, :], lhs[:, :], W_sb[:, 1024:1088], start=True, stop=True)

            b = None
            s_prev = None   # AP of s for final output
            n2_prev = None
            f_prev = None
            for it in range(ITERS):
                if it == 0:
                    s_ap = s1[:, :]          # [128, 64] (d, o) in PSUM
                else:
                    # softmax over o:  e = exp(b); Z[i] = sum_o e; c = e / Z
                    e = smpool.tile([P, 256], F32, tag="e")
                    nc.scalar.activation(e[:, :], b[:, :], AF.Exp)
                    Z = smpool.tile([P, O], F32, tag="Z")
                    nc.vector.reduce_sum(
                        Z[:, :],
                        e[:, :].rearrange("p (o i) -> p i o", o=O, i=I),
                        axis=AX.X)
                    zi = smpool.tile([P, O], F32, tag="zi")
                    nc.vector.reciprocal(zi[:, :], Z[:, :])
                    c = smpool.tile([P, 256], F32, tag="c")
                    nc.vector.tensor_tensor(
                        out=c[:, :].rearrange("p (o i) -> p o i", o=O, i=I),
                        in0=e[:, :].rearrange("p (o i) -> p o i", o=O, i=I),
                        in1=_strided(zi[:, :], 0, [[0, O], [1, I]]),
                        op=OP.mult)
                    # s[d,o] = sum_i c[o,i]*votes[d,o,i]
                    t1 = bigpool.tile([P, 1024], F32, tag="t1")
                    nc.vector.tensor_tensor(
                        out=t1[:, :].rearrange("p (d oi) -> p d oi", d=4, oi=256),
                        in0=votes[:, :].rearrange("p (d oi) -> p d oi", d=4, oi=256),
                        in1=_strided(c[:, :], 0, [[0, 4], [1, 256]]),
                        op=OP.mult)
                    s_ap_t = smpool.tile([P, 64], F32, tag="s")
                    nc.vector.reduce_sum(
                        s_ap_t[:, :],
                        t1[:, :].rearrange("p (do i) -> p do i", do=64, i=I),
                        axis=AX.X)
                    s_ap = s_ap_t[:, :]

                # ---- squash factor: f = n2 / ((1+n2) sqrt(n2+eps)) ----
                sq = smpool.tile([P, 64], F32, tag="sq")
                nc.scalar.activation(sq[:, :], s_ap, AF.Square)
                n2 = smpool.tile([P, O], F32, tag="n2")
                nc.vector.reduce_sum(
                    n2[:, :],
                    sq[:, :].rearrange("p (d o) -> p o d", d=4, o=O),
                    axis=AX.X)
                u = smpool.tile([P, O], F32, tag="u")
                nc.vector.tensor_scalar_add(u[:, :], n2[:, :], 1.0)
                w_ = smpool.tile([P, O], F32, tag="w")
                nc.vector.reciprocal(w_[:, :], u[:, :])
                ln_ = smpool.tile([P, O], F32, tag="ln")
                nc.scalar.activation(ln_[:, :], n2[:, :], AF.Ln, bias=eps_sb[:, :])
                r = smpool.tile([P, O], F32, tag="r")
                nc.scalar.activation(r[:, :], ln_[:, :], AF.Exp, scale=-0.5)
                f = smpool.tile([P, O], F32, tag="f")
                nc.vector.tensor_tensor(out=f[:, :], in0=n2[:, :], in1=w_[:, :], op=OP.mult)
                nc.vector.tensor_tensor(out=f[:, :], in0=f[:, :], in1=r[:, :], op=OP.mult)

                if it < ITERS - 1:
                    # v = s * f  (broadcast f over d)
                    v = smpool.tile([P, 64], F32, tag="v")
                    nc.vector.tensor_tensor(
                        out=v[:, :].rearrange("p (d o) -> p d o", d=4, o=O),
                        in0=s_ap.rearrange("p (d o) -> p d o", d=4, o=O),
                        in1=_strided(f[:, :], 0, [[0, 4], [1, O]]),
                        op=OP.mult)
                    # delta_b[o,i] = sum_d votes[d,o,i] * v[d,o]
                    t2 = bigpool.tile([P, 1024], F32, tag="t2")
                    nc.vector.tensor_tensor(
                        out=t2[:, :].rearrange("p (d o i) -> p d o i", d=4, o=O, i=I),
                        in0=votes[:, :].rearrange("p (d o i) -> p d o i", d=4, o=O, i=I),
                        in1=_strided(v[:, :], 0, [[16, 4], [1, O], [0, I]]),
                        op=OP.mult)
                    db = smpool.tile([P, 256], F32, tag="db")
                    nc.vector.reduce_sum(
                        db[:, :],
                        t2[:, :].rearrange("p (d oi) -> p oi d", d=4, oi=256),
                        axis=AX.X)
                    if b is None:
                        b = db
                    else:
                        b2 = smpool.tile([P, 256], F32, tag="b2")
                        nc.vector.tensor_tensor(out=b2[:, :], in0=b[:, :], in1=db[:, :], op=OP.add)
                        b = b2
                else:
                    s_prev, n2_prev, f_prev = s_ap, n2, (f, r)

            # ---- activation gate ----
            #   norm ~= n2 * rsqrt(n2+eps);  z = beta*norm + (alpha+bias)
            #   a = 1/(1+exp(-z));  out = s * (f*a)  broadcast over d
            f, r = f_prev
            z = smpool.tile([P, O], F32, tag="z")
            nc.vector.tensor_tensor(out=z[:, :], in0=n2_prev[:, :], in1=r[:, :], op=OP.mult)
            nc.vector.tensor_tensor(out=z[:, :], in0=z[:, :], in1=bb_sb[:, :], op=OP.mult)
            nc.vector.tensor_tensor(out=z[:, :], in0=z[:, :], in1=ab_sb[:, :], op=OP.add)
            zc = smpool.tile([P, O], F32, tag="zc")
            nc.vector.tensor_scalar(out=zc[:, :], in0=z[:, :], scalar1=-87.0,
                                    scalar2=87.0, op0=OP.max, op1=OP.min)
            m = smpool.tile([P, O], F32, tag="m")
            nc.scalar.activation(m[:, :], zc[:, :], AF.Exp, scale=-1.0)
            den = smpool.tile([P, O], F32, tag="den")
            nc.vector.tensor_scalar_add(den[:, :], m[:, :], 1.0)
            a = smpool.tile([P, O], F32, tag="a")
            nc.vector.reciprocal(a[:, :], den[:, :])
            g = smpool.tile([P, O], F32, tag="g")
            nc.vector.tensor_tensor(out=g[:, :], in0=f[:, :], in1=a[:, :], op=OP.mult)
            # out[(o,d)] = s[(d,o)] * g[o]
            ysl = ys[j][:, :]
            out_ap = bass.AP(ysl.tensor, ysl.offset + tt * 64,
                             [list(ysl.ap[0]), [1, 4], [4, O]])
            nc.vector.tensor_tensor(
                out=out_ap,
                in0=s_prev.rearrange("p (d o) -> p d o", d=4, o=O),
                in1=_strided(g[:, :], 0, [[0, 4], [1, O]]),
                op=OP.mult)

            if tt == TPC - 1:
                rows = y_d[j * TPC * P:(j + 1) * TPC * P, :]
                nc.sync.dma_start(
                    out=rows.rearrange("(t p) f -> p t f", p=P),
                    in_=ys[j][:, :].rearrange("p (t f) -> p t f", f=64),
                )
    _fixup_bir_for_walrus(nc)
    return nc


_NC_CACHE = None


def _get_nc():
    global _NC_CACHE
    if _NC_CACHE is None:
        _NC_CACHE = _build_nc()
    return _NC_CACHE


class _Runner:
    """Cached shard_map-jitted executor over the 8 cores (mirrors
    bass2jax.run_bass_via_pjrt, but built once and reused)."""

    def __init__(self):
        import jax
        from jax.experimental.shard_map import shard_map
        from jax.sharding import Mesh, PartitionSpec, NamedSharding
        from concourse.bass2jax import (
            _bass_exec_p, install_neuronx_cc_hook, partition_id_tensor)

        install_neuronx_cc_hook()
        nc = _get_nc()
        in_names, out_names, out_avals = [], [], []
        import concourse.mybir as _mb
        pid_name = nc.partition_id_tensor.name if nc.partition_id_tensor else None
        for alloc in nc.m.functions[0].allocations:
            if not isinstance(alloc, _mb.MemoryLocationSet):
                continue
            name = alloc.memorylocations[0].name
            if alloc.kind == "ExternalInput":
                if name != pid_name:
                    in_names.append(name)
            elif alloc.kind == "ExternalOutput":
                out_names.append(name)
                out_avals.append(jax.core.ShapedArray(
                    tuple(alloc.tensor_shape), _mb.dt.np(alloc.dtype)))
        self.in_names, self.out_names, self.out_avals = in_names, out_names, out_avals
        n_params, n_outs = len(in_names), len(out_names)
        all_names = list(in_names) + list(out_names)
        if pid_name is not None:
            all_names.append(pid_name)

        def _body(*args):
            operands = list(args)
            if pid_name is not None:
                operands.append(partition_id_tensor())
            outs = _bass_exec_p.bind(
                *operands,
                out_avals=tuple(out_avals),
                in_names=tuple(all_names),
                out_names=tuple(out_names),
                lowering_input_output_aliases=(),
                sim_require_finite=True,
                sim_require_nnan=True,
                nc=nc,
            )
            return tuple(outs)

        devices = jax.devices()[:NCORES]
        self.mesh = Mesh(np.asarray(devices), ("core",))
        self.pspec = PartitionSpec("core")
        self.sharding = NamedSharding(self.mesh, self.pspec)
        in_specs = (self.pspec,) * (n_params + n_outs)
        out_specs = (self.pspec,) * n_outs
        self.fn = jax.jit(
            shard_map(_body, mesh=self.mesh, in_specs=in_specs,
                      out_specs=out_specs, check_rep=False),
            donate_argnums=tuple(range(n_params, n_params + n_outs)),
            keep_unused=True,
        )
        self._jax = jax

    def zeros(self):
        return [np.zeros((NCORES * a.shape[0], *a.shape[1:]), a.dtype)
                for a in self.out_avals]

    def run(self, concat_inputs):
        outs = self.fn(*concat_inputs, *self.zeros())
        return [np.asarray(o) for o in outs]


_RUNNER = None


def _get_runner():
    global _RUNNER
    if _RUNNER is None:
        _RUNNER = _Runner()
    return _RUNNER


def _prep_inputs(x, quats, scale, trans, bias=None, beta=None, alpha=None):
    """Concatenated (over cores, axis 0) input list in runner order."""
    x = np.asarray(x, np.float32)
    W = _build_W(np.asarray(quats), np.asarray(scale), np.asarray(trans))
    bb = np.tile(np.asarray(beta, np.float32)[None, :], (P, 1))
    ab = np.tile((np.asarray(alpha, np.float32)
                  + np.asarray(bias, np.float32))[None, :], (P, 1))
    ident = np.eye(P, dtype=np.float32)
    x_aug = np.empty((N, 65), np.float32)
    x_aug[:, :64] = x.reshape(N, 64)
    x_aug[:, 64] = 1.0
    per_core = {
        "x": x_aug,                                  # already n-major
        "W": np.concatenate([W] * NCORES, axis=0),
        "bb": np.concatenate([bb] * NCORES, axis=0),
        "ab": np.concatenate([ab] * NCORES, axis=0),
        "ident": np.concatenate([ident] * NCORES, axis=0),
    }
    r = _get_runner()
    return [per_core[name] for name in r.in_names]


def kernel(x, quats, scale, trans, bias, beta, alpha):
    r = _get_runner()
    concat_in = _prep_inputs(x, quats, scale, trans,
                             bias=bias, beta=beta, alpha=alpha)
    outs = r.run(concat_in)
    y = outs[r.out_names.index("y")]
    return np.ascontiguousarray(y.reshape(N, O, 4).astype(np.float32))


# revision 41
# speedup vs baseline: 27.5794x; 27.5794x over previous
"""Trainium2 Bass kernel for quaternion capsule routing layer.

Math (reference):
  qn = normalize(quats); votes[n,o,i,:] = scale[o,i]*H(qn[o,i], x[n,i]) + (0,trans)
  3 iterations of dynamic routing (softmax over o, weighted vote sum, squash,
  agreement update), then sigmoid-gated output poses.

Strategy (8 cores, data-parallel over n):
  - votes are a LINEAR map of x: host folds rotation+scale+translation into
    W [65, 1088] (64 x-features + ones row).  Columns 0..1023 = votes in
    (d, o, i) order, columns 1024..1087 = iteration-1 s (c is uniform 1/16).
  - Per 128-capsule tile: PE transposes x, float32r matmuls produce votes in
    PSUM; routing runs on DVE (custom fused multiply+prefix-scan op for the
    two big einsums, segment sums extracted by strided subtracts) and ACT
    (exp/ln only -> single activation table set, no table switches).
"""

import numpy as np

import concourse.bass as bass
import concourse.mybir as mybir
import concourse.dve_ops as dve_ops
from concourse.dve_spec import Spec, Src0, Src1, scan, AluOp, lower, _has_src1
from concourse.dve_uop import DveOpSpec
from concourse.tile import TileContext
from concourse import bass_utils

F32 = mybir.dt.float32
F32R = mybir.dt.float32r
BF16 = mybir.dt.bfloat16
AX = mybir.AxisListType
OP = mybir.AluOpType
AF = mybir.ActivationFunctionType

N, I, O, ITERS = 65536, 16, 16, 3
EPS = 1e-8
NCORES = 8
P = 128
N_CORE = N // NCORES            # 8192
NT = N_CORE // P                # 64 tiles per core
NCHUNK = 4                      # x/y staged in 4 DMA chunks
TPC = NT // NCHUNK              # tiles per chunk


# --------------------------------------------------------------------------
# custom DVE op: out[p,k] = cumsum_k(in0[p,k] * in1[p,k])   (fp32, inclusive)
# --------------------------------------------------------------------------
def _mulscan_ref(in0, in1, s0, s1, imm2):
    p = in0.astype(np.float32) * np.asarray(in1, dtype=np.float32)
    flat = p.reshape(p.shape[0], -1)
    return np.cumsum(flat, axis=1, dtype=np.float32).reshape(in0.shape)


def _register_mulscan():
    name = "CAPS_MULSCAN_V1"
    for op in dve_ops.OPS:
        if op.name == name:
            return op
    row = dve_ops._CUSTOM_DVE_ROW_BASE + len(dve_ops.OPS)
    dve_ops._SUB_OPCODE_FOR_NAME[name] = row
    spec = Spec(body=scan(AluOp.ADD, Src0 * Src1), reference=_mulscan_ref)
    shas = {}
    for ver in ("v3", "v4"):
        try:
            s = DveOpSpec(name=name, opcode=row, uops=lower(spec, ver=ver),
                          rd1_en=_has_src1(spec))
            shas[ver] = s.sha(ver)
        except Exception:
            pass
    op = dve_ops.DveOp(name, spec, subdim=False, uops_sha=shas)
    dve_ops.OPS.append(op)
    dve_ops.CUSTOM_DVE_SPECS[name] = spec
    return op


MULSCAN = _register_mulscan()


# --------------------------------------------------------------------------
# host-side parameter folding
# --------------------------------------------------------------------------
def _build_W(quats, scale, trans):
    """W [65, 1088] f32.  Rows (i*4+e) for e in 0..3 plus ones-row 64.
    Cols 0..1023: votes, col j = d*256 + o*16 + i.
    Cols 1024..1087: iter-1 s (=mean of votes over i), col 1024 + d*16 + o."""
    q = quats.astype(np.float64)
    qn = q / np.sqrt((q * q).sum(-1, keepdims=True) + EPS)
    w, x, y, z = qn[..., 0], qn[..., 1], qn[..., 2], qn[..., 3]
    # Hamilton left-multiplication matrix M[d, e]: (q (x) r)_d = sum_e M_de r_e
    M = np.stack([
        np.stack([w, -x, -y, -z], -1),
        np.stack([x,  w, -z,  y], -1),
        np.stack([y,  z,  w, -x], -1),
        np.stack([z, -y,  x,  w], -1),
    ], -2)                                    # [O, I, d, e]
    A = scale.astype(np.float64)[..., None] * M     # [O, I, d, e]
    t = np.concatenate([np.zeros(trans.shape[:-1] + (1,)),
                        trans.astype(np.float64)], -1)   # [O, I, d]

    W = np.zeros((65, 1088), np.float64)
    # votes block
    Wv = W[:, :1024].reshape(65, 4, O, I)       # [row, d, o, i]
    for i in range(I):
        # rows i*4+e  ->  cols (d, o, i)
        Wv[i * 4:(i + 1) * 4, :, :, i] = A[:, i, :, :].transpose(2, 1, 0)  # [e,d,o]? fix below
    # careful: A[o,i,d,e] -> want Wv[i*4+e, d, o, i] = A[o,i,d,e]
    # A[:, i, :, :] is [o, d, e]; transpose(2,1,0) -> [e, d, o]  (correct)
    Wv[64, :, :, :] = t.transpose(2, 0, 1)      # [d, o, i] <- t[o,i,d]
    # s1 block: s1[d,o] = (1/16) * sum_i votes[d,o,i]
    Ws = W[:, 1024:].reshape(65, 4, O)
    for i in range(I):
        Ws[i * 4:(i + 1) * 4, :, :] += A[:, i, :, :].transpose(2, 1, 0) / I
    Ws[64, :, :] += t.transpose(2, 0, 1).sum(-1) / I
    return np.ascontiguousarray(W, dtype=np.float32)


# --------------------------------------------------------------------------
# device kernel
# --------------------------------------------------------------------------
def _strided(ap, off, dims):
    """AP at element offset `off` past `ap`'s origin with free dims
    [[step, count], ...] (partition dim copied from ap)."""
    return bass.AP(ap.tensor, ap.offset + off, [list(ap.ap[0])] + [list(d) for d in dims])


def _fixup_bir_for_walrus(nc):
    """Adapt Tile/bass output to this container's walrus build:
    1. Every ISA struct here has a single sync-wait slot; Tile can emit
       several waits on one instruction.  Move all-but-one wait onto NoOps
       inserted just before it in the same engine stream (equivalent:
       waits hold monotonically within an execution phase).
    2. EVENT_SEMAPHORE_RANGE_CLEAR (opcode 176) is unknown to this walrus;
       replace with per-semaphore writes of 0."""
    import bass_rust as _br
    cnt = 0
    for blk in nc.m.functions[0].blocks:
        out = []
        changed = False
        for ins in blk.instructions:
            si = ins.sync_info
            if si is not None and len(si.on_wait) > 1:
                waits = list(si.on_wait)
                for w in waits[:-1]:
                    cnt += 1
                    nop = mybir.InstNoOp(
                        name=f"I-wsplit-{cnt}", engine=ins.engine,
                        text_hint="wsplit", bass_nofuse=True,
                        ins=[], outs=[],
                        sync_info=_br.SyncInfo(on_wait=[w], on_update=[]))
                    out.append(nop)
                ins.sync_info = _br.SyncInfo(
                    on_wait=[waits[-1]], on_update=list(si.on_update))
                changed = True
            if (type(ins).__name__ == "InstISA"
                    and getattr(ins, "ant_dict", None)
                    and ins.ant_dict.get("header", {}).get("opcode") == 176):
                lo = ins.ant_dict["range_first"]
                hi = ins.ant_dict["range_last"]
                base_si = ins.sync_info
                for k, sem in enumerate(range(lo, hi + 1)):
                    cnt += 1
                    upd = _br.SyncUpdate(
                        sync_type="semaphore", id=sem,
                        update_mode="sem-wr-imm", update_value=0)
                    ev = mybir.InstEventSemaphore(
                        name=f"I-semclr-{cnt}", engine=ins.engine,
                        ins=[], outs=[],
                        sync_info=_br.SyncInfo(
                            on_wait=list(base_si.on_wait) if (k == 0 and base_si) else [],
                            on_update=[upd]))
                    out.append(ev)
                changed = True
                continue
            out.append(ins)
        if changed:
            blk.instructions = out
    return cnt


GP_OFFLOAD = False  # run SBUF-only small elementwise ops on GpSimd (measured slower)
BF16_VOTES = False  # stage votes in SBUF as bf16 (no measured speedup; hurts accuracy)


def _build_nc():
    nc = bass.Bass(trn_type="TRN2")
    x_d = nc.dram_tensor("x", [N_CORE, 65], F32, kind="ExternalInput")
    W_d = nc.dram_tensor("W", [65, 1088], F32, kind="ExternalInput")
    bb_d = nc.dram_tensor("bb", [P, O], F32, kind="ExternalInput")   # beta bcast
    ab_d = nc.dram_tensor("ab", [P, O], F32, kind="ExternalInput")   # alpha+bias bcast
    id_d = nc.dram_tensor("ident", [P, P], F32, kind="ExternalInput")
    y_d = nc.dram_tensor("y", [N_CORE, 64], F32, kind="ExternalOutput")

    with TileContext(nc) as tc, \
         tc.tile_pool(name="const", bufs=1) as cpool, \
         tc.tile_pool(name="stage", bufs=1) as spool, \
         tc.tile_pool(name="lhs", bufs=3) as lpool, \
         tc.tile_pool(name="big", bufs=2) as bigpool, \
         tc.tile_pool(name="vbf", bufs=3) as vbfpool, \
         tc.tile_pool(name="sm", bufs=3) as smpool, \
         tc.tile_pool(name="pv", bufs=2, space="PSUM") as pv, \
         tc.tile_pool(name="px", bufs=2, space="PSUM") as px, \
         tc.tile_pool(name="ps1", bufs=2, space="PSUM") as ps1:

        W_sb = cpool.tile([65, 1088], F32, tag="W")
        id_sb = cpool.tile([P, P], F32, tag="ident")
        bb_sb = cpool.tile([P, O], F32, tag="bb")
        ab_sb = cpool.tile([P, O], F32, tag="ab")
        eps_sb = cpool.tile([P, 1], F32, tag="eps")
        nc.vector.memset(eps_sb[:, :], EPS)
        nc.sync.dma_start(out=W_sb[:, :], in_=W_d[:, :])
        nc.sync.dma_start(out=id_sb[:, :], in_=id_d[:, :])
        nc.sync.dma_start(out=bb_sb[:, :], in_=bb_d[:, :])
        nc.sync.dma_start(out=ab_sb[:, :], in_=ab_d[:, :])

        xs = spool.tile([P, NT * 65], F32, tag="xs")
        nc.sync.dma_start(
            out=xs[:, :].rearrange("p (t f) -> p t f", f=65),
            in_=x_d[:, :].rearrange("(t p) f -> p t f", p=P),
        )
        ys = []
        for j in range(NCHUNK):
            ys_j = spool.tile([P, TPC * 64], F32, tag=f"ys{j}")
            ys.append(ys_j)

        # Prologue: PE ops absorbing one DMA-lane wait each (the LDWEIGHTS
        # struct supports a single sync wait), so steady-state matmuls only
        # ever wait on the DVE semaphore.
        pa = px.tile([P, P], F32, tag="xt")
        nc.tensor.transpose(pa[:, :], id_sb[:, :], id_sb[:, :])
        pb = px.tile([P, P], F32, tag="xt")
        nc.tensor.transpose(pb[:, :], W_sb[:, 0:P], id_sb[0:65, :])
        pc = px.tile([P, P], F32, tag="xt")
        nc.tensor.transpose(pc[:65, :], xs[:, 0:65], id_sb[:, :])

        for t in range(NT):
            j, tt = divmod(t, TPC)
            # ---- PE: transpose x tile, matmul votes + s1 ----
            xt = px.tile([65, P], F32, tag="xt")
            nc.tensor.transpose(xt[:, :], xs[:, t * 65:(t + 1) * 65], id_sb[:, :])
            lhs = lpool.tile([65, P], F32, tag="lhs")
            nc.vector.tensor_copy(lhs[:, :], xt[:, :])

            votes = pv.tile([P, 1024], F32, tag="votes")
            s1 = ps1.tile([P, 64], F32, tag="s1")
            nc.tensor.matmul(votes[:, 0:512], lhs[:, :], W_sb[:, 0:512], start=True, stop=True)
            nc.tensor.matmul(votes[:, 512:1024], lhs[:, :], W_sb[:, 512:1024], start=True, stop=True)
            nc.tensor.matmul(s1[:, :], lhs[:, :], W_sb[:, 1024:1088], start=True, stop=True)

            if BF16_VOTES:
                vbf = vbfpool.tile([P, 1024], BF16, tag="vbf")
                nc.scalar.copy(vbf[:, :], votes[:, :])
                vsrc = vbf[:, :]
                RDT = BF16
            else:
                vsrc = votes[:, :]
                RDT = F32

            b = None
            s_prev = None   # AP of s for final output
            n2_prev = None
            f_prev = None
            for it in range(ITERS):
                if it == 0:
                    s_ap = s1[:, :]          # [128, 64] (d, o) in PSUM
                else:
                    # softmax over o:  e = exp(b); Z[i] = sum_o e; c = e / Z
                    e = smpool.tile([P, 256], F32, tag="e")
                    nc.scalar.activation(e[:, :], b[:, :], AF.Exp)
                    Z = smpool.tile([P, O], F32, tag="Z")
                    nc.vector.reduce_sum(
                        Z[:, :],
                        e[:, :].rearrange("p (o i) -> p i o", o=O, i=I),
                        axis=AX.X)
                    zi = smpool.tile([P, O], F32, tag="zi")
                    nc.vector.reciprocal(zi[:, :], Z[:, :])
                    c = smpool.tile([P, 256], RDT, tag="c")
                    ceng = nc.gpsimd if GP_OFFLOAD else nc.vector
                    ceng.tensor_tensor(
                        out=c[:, :].rearrange("p (o i) -> p o i", o=O, i=I),
                        in0=e[:, :].rearrange("p (o i) -> p o i", o=O, i=I),
                        in1=_strided(zi[:, :], 0, [[0, O], [1, I]]),
                        op=OP.mult)
                    # s[d,o] = sum_i c[o,i]*votes[d,o,i]
                    t1 = bigpool.tile([P, 1024], RDT, tag="t1")
                    nc.vector.tensor_tensor(
                        out=t1[:, :].rearrange("p (d oi) -> p d oi", d=4, oi=256),
                        in0=vsrc.rearrange("p (d oi) -> p d oi", d=4, oi=256),
                        in1=_strided(c[:, :], 0, [[0, 4], [1, 256]]),
                        op=OP.mult)
                    s_ap_t = smpool.tile([P, 64], F32, tag="s")
                    nc.vector.reduce_sum(
                        s_ap_t[:, :],
                        t1[:, :].rearrange("p (do i) -> p do i", do=64, i=I),
                        axis=AX.X)
                    s_ap = s_ap_t[:, :]

                # ---- squash factor: f = n2 / ((1+n2) sqrt(n2+eps)) ----
                sq = smpool.tile([P, 64], F32, tag="sq")
                nc.scalar.activation(sq[:, :], s_ap, AF.Square)
                n2 = smpool.tile([P, O], F32, tag="n2")
                nc.vector.reduce_sum(
                    n2[:, :],
                    sq[:, :].rearrange("p (d o) -> p o d", d=4, o=O),
                    axis=AX.X)
                u = smpool.tile([P, O], F32, tag="u")
                nc.vector.tensor_scalar_add(u[:, :], n2[:, :], 1.0)
                w_ = smpool.tile([P, O], F32, tag="w")
                nc.vector.reciprocal(w_[:, :], u[:, :])
                ln_ = smpool.tile([P, O], F32, tag="ln")
                nc.scalar.activation(ln_[:, :], n2[:, :], AF.Ln, bias=eps_sb[:, :])
                r = smpool.tile([P, O], F32, tag="r")
                nc.scalar.activation(r[:, :], ln_[:, :], AF.Exp, scale=-0.5)
                f = smpool.tile([P, O], F32, tag="f")
                feng = nc.gpsimd if GP_OFFLOAD else nc.vector
                feng.tensor_tensor(out=f[:, :], in0=n2[:, :], in1=w_[:, :], op=OP.mult)
                feng.tensor_tensor(out=f[:, :], in0=f[:, :], in1=r[:, :], op=OP.mult)

                if it < ITERS - 1:
                    # v = s * f  (broadcast f over d); s in PSUM on iter 0
                    v = smpool.tile([P, 64], RDT, tag="v")
                    veng = nc.gpsimd if (GP_OFFLOAD and it > 0) else nc.vector
                    veng.tensor_tensor(
                        out=v[:, :].rearrange("p (d o) -> p d o", d=4, o=O),
                        in0=s_ap.rearrange("p (d o) -> p d o", d=4, o=O),
                        in1=_strided(f[:, :], 0, [[0, 4], [1, O]]),
                        op=OP.mult)
                    # delta_b[o,i] = sum_d votes[d,o,i] * v[d,o]
                    t2 = bigpool.tile([P, 1024], RDT, tag="t2")
                    nc.vector.tensor_tensor(
                        out=t2[:, :].rearrange("p (d o i) -> p d o i", d=4, o=O, i=I),
                        in0=vsrc.rearrange("p (d o i) -> p d o i", d=4, o=O, i=I),
                        in1=_strided(v[:, :], 0, [[16, 4], [1, O], [0, I]]),
                        op=OP.mult)
                    db = smpool.tile([P, 256], F32, tag="db")
                    nc.vector.reduce_sum(
                        db[:, :],
                        t2[:, :].rearrange("p (d oi) -> p oi d", d=4, oi=256),
                        axis=AX.X)
                    if b is None:
                        b = db
                    else:
                        b2 = smpool.tile([P, 256], F32, tag="b2")
                        beng = nc.gpsimd if GP_OFFLOAD else nc.vector
                        beng.tensor_tensor(out=b2[:, :], in0=b[:, :], in1=db[:, :], op=OP.add)
                        b = b2
                else:
                    s_prev, n2_prev, f_prev = s_ap, n2, (f, r)

            # ---- activation gate ----
            #   norm ~= n2 * rsqrt(n2+eps);  z = beta*norm + (alpha+bias)
            #   a = 1/(1+exp(-z));  out = s * (f*a)  broadcast over d
            f, r = f_prev
            zeng = nc.gpsimd if GP_OFFLOAD else nc.vector
            z = smpool.tile([P, O], F32, tag="z")
            zeng.tensor_tensor(out=z[:, :], in0=n2_prev[:, :], in1=r[:, :], op=OP.mult)
            zeng.tensor_tensor(out=z[:, :], in0=z[:, :], in1=bb_sb[:, :], op=OP.mult)
            zeng.tensor_tensor(out=z[:, :], in0=z[:, :], in1=ab_sb[:, :], op=OP.add)
            zc = smpool.tile([P, O], F32, tag="zc")
            nc.vector.tensor_scalar(out=zc[:, :], in0=z[:, :], scalar1=-87.0,
                                    scalar2=87.0, op0=OP.max, op1=OP.min)
            m = smpool.tile([P, O], F32, tag="m")
            nc.scalar.activation(m[:, :], zc[:, :], AF.Exp, scale=-1.0)
            den = smpool.tile([P, O], F32, tag="den")
            nc.vector.tensor_scalar_add(den[:, :], m[:, :], 1.0)
            a = smpool.tile([P, O], F32, tag="a")
            nc.vector.reciprocal(a[:, :], den[:, :])
            g = smpool.tile([P, O], F32, tag="g")
            nc.vector.tensor_tensor(out=g[:, :], in0=f[:, :], in1=a[:, :], op=OP.mult)
            # out[(o,d)] = s[(d,o)] * g[o]
            ysl = ys[j][:, :]
            out_ap = bass.AP(ysl.tensor, ysl.offset + tt * 64,
                             [list(ysl.ap[0]), [1, 4], [4, O]])
            nc.vector.tensor_tensor(
                out=out_ap,
                in0=s_prev.rearrange("p (d o) -> p d o", d=4, o=O),
                in1=_strided(g[:, :], 0, [[0, 4], [1, O]]),
                op=OP.mult)

            if tt == TPC - 1:
                rows = y_d[j * TPC * P:(j + 1) * TPC * P, :]
                nc.sync.dma_start(
                    out=rows.rearrange("(t p) f -> p t f", p=P),
                    in_=ys[j][:, :].rearrange("p (t f) -> p t f", f=64),
                )
    _fixup_bir_for_walrus(nc)
    return nc


_NC_CACHE = None


def _get_nc():
    global _NC_CACHE
    if _NC_CACHE is None:
        _NC_CACHE = _build_nc()
    return _NC_CACHE


class _Runner:
    """Cached shard_map-jitted executor over the 8 cores (mirrors
    bass2jax.run_bass_via_pjrt, but built once and reused)."""

    def __init__(self):
        import jax
        from jax.experimental.shard_map import shard_map
        from jax.sharding import Mesh, PartitionSpec, NamedSharding
        from concourse.bass2jax import (
            _bass_exec_p, install_neuronx_cc_hook, partition_id_tensor)

        install_neuronx_cc_hook()
        nc = _get_nc()
        in_names, out_names, out_avals = [], [], []
        import concourse.mybir as _mb
        pid_name = nc.partition_id_tensor.name if nc.partition_id_tensor else None
        for alloc in nc.m.functions[0].allocations:
            if not isinstance(alloc, _mb.MemoryLocationSet):
                continue
            name = alloc.memorylocations[0].name
            if alloc.kind == "ExternalInput":
                if name != pid_name:
                    in_names.append(name)
            elif alloc.kind == "ExternalOutput":
                out_names.append(name)
                out_avals.append(jax.core.ShapedArray(
                    tuple(alloc.tensor_shape), _mb.dt.np(alloc.dtype)))
        self.in_names, self.out_names, self.out_avals = in_names, out_names, out_avals
        n_params, n_outs = len(in_names), len(out_names)
        all_names = list(in_names) + list(out_names)
        if pid_name is not None:
            all_names.append(pid_name)

        def _body(*args):
            operands = list(args)
            if pid_name is not None:
                operands.append(partition_id_tensor())
            outs = _bass_exec_p.bind(
                *operands,
                out_avals=tuple(out_avals),
                in_names=tuple(all_names),
                out_names=tuple(out_names),
                lowering_input_output_aliases=(),
                sim_require_finite=True,
                sim_require_nnan=True,
                nc=nc,
            )
            return tuple(outs)

        devices = jax.devices()[:NCORES]
        self.mesh = Mesh(np.asarray(devices), ("core",))
        self.pspec = PartitionSpec("core")
        self.sharding = NamedSharding(self.mesh, self.pspec)
        in_specs = (self.pspec,) * (n_params + n_outs)
        out_specs = (self.pspec,) * n_outs
        self.fn = jax.jit(
            shard_map(_body, mesh=self.mesh, in_specs=in_specs,
                      out_specs=out_specs, check_rep=False),
            donate_argnums=tuple(range(n_params, n_params + n_outs)),
            keep_unused=True,
        )
        self._jax = jax

    def zeros(self):
        """Donated output buffers, filled device-side (no host upload)."""
        import jax
        import jax.numpy as jnp
        if not hasattr(self, "_zfn"):
            avals = self.out_avals
            self._zfn = jax.jit(
                lambda: tuple(
                    jnp.zeros((NCORES * a.shape[0], *a.shape[1:]), a.dtype)
                    for a in avals),
                out_shardings=tuple(self.sharding for _ in avals))
        return list(self._zfn())

    def run(self, concat_inputs):
        outs = self.fn(*concat_inputs, *self.zeros())
        return [np.asarray(o) for o in outs]


_RUNNER = None


def _get_runner():
    global _RUNNER
    if _RUNNER is None:
        _RUNNER = _Runner()
    return _RUNNER


def _prep_inputs(x, quats, scale, trans, bias=None, beta=None, alpha=None):
    """Concatenated (over cores, axis 0) input list in runner order."""
    x = np.asarray(x, np.float32)
    W = _build_W(np.asarray(quats), np.asarray(scale), np.asarray(trans))
    bb = np.tile(np.asarray(beta, np.float32)[None, :], (P, 1))
    ab = np.tile((np.asarray(alpha, np.float32)
                  + np.asarray(bias, np.float32))[None, :], (P, 1))
    ident = np.eye(P, dtype=np.float32)
    x_aug = np.empty((N, 65), np.float32)
    x_aug[:, :64] = x.reshape(N, 64)
    x_aug[:, 64] = 1.0
    per_core = {
        "x": x_aug,                                  # already n-major
        "W": np.concatenate([W] * NCORES, axis=0),
        "bb": np.concatenate([bb] * NCORES, axis=0),
        "ab": np.concatenate([ab] * NCORES, axis=0),
        "ident": np.concatenate([ident] * NCORES, axis=0),
    }
    r = _get_runner()
    return [per_core[name] for name in r.in_names]


def kernel(x, quats, scale, trans, bias, beta, alpha):
    r = _get_runner()
    concat_in = _prep_inputs(x, quats, scale, trans,
                             bias=bias, beta=beta, alpha=alpha)
    outs = r.run(concat_in)
    y = outs[r.out_names.index("y")]
    return np.ascontiguousarray(y.reshape(N, O, 4).astype(np.float32))
